# revision 5
# baseline (speedup 1.0000x reference)
"""Trainium2 Bass kernel for BigraphLightModel (two-stage LightGCN).

Strategy (8 NeuronCores, SPMD), v2 — gather + TensorE segment-sum:
- Nodes of each graph are partitioned contiguously across the 8 cores
  (core k owns rows [k*N/8, (k+1)*N/8)); edges live on the core that owns
  their dst node. Each layer the full embedding table is rebuilt on every
  core by an AllGather (HBM->HBM), and each core dma_gathers the source
  rows of its edges from its local copy.
- The scatter-add is NOT done with dma_scatter_add (SWDGE descriptor
  generation on the Q7 cores was the 12ms bottleneck of v1). Instead,
  edges are laid out sorted by (src-window, dst-tile) where dst-tile is a
  128-row block of the core's node slice. For each 128-slot chunk the
  kernel builds a one-hot selection matrix S[slot, dst_local] on the
  Vector engine (is_equal against an iota row) and computes
  psum[dst, :] += S.T @ (norm * gathered) on the Tensor engine. PSUM
  tiles accumulate over a (window, tile) group and are added into an
  SBUF accumulator holding the core's [R, D] output slice.
- Slot layout is identical on all 8 cores (a (window,tile) group gets
  max-over-cores chunks, padded with norm=0 slots), so a single SPMD
  program works for every core.
- The per-layer output slice goes acc -> HBM -> AllGather -> next table.
  The item-block splice emb_uiu[item_idx] = h_ii is the AllGather of
  h_ii written into rows [N_USERS:N_USERS+N_II) of the uiu table.
"""
import os
import sys

for _p in ("/opt/trn_rl_repo",):
    if _p not in sys.path and os.path.isdir(_p):
        sys.path.insert(0, _p)

import numpy as np
import ml_dtypes

# -------------------- problem constants --------------------
N_II = 100000
N_UIU = 200000
D = 64
L = 3                   # layers per graph
N_CORES = 8
N_USERS = 100000
WIN = 32768             # int16 gather window (rows)
CCH = 64                # chunks per gather call (64*128 = 8192 slots)
GTILES = 8              # dst tiles per PSUM group (8*64 fp32 = one bank)

_LAST_RESULT = None     # test harness reads this for profiling info


# -------------------- host-side graph planning --------------------

def _plan_graph(src, dst, ew, n_nodes, n_cores=N_CORES, win=WIN):
    """Slot layout for one graph: slots sorted by (window, dst-tile),
    each (window, tile) padded to a uniform chunk count across cores."""
    src = np.asarray(src).astype(np.int64)
    dst = np.asarray(dst).astype(np.int64)
    ew = np.asarray(ew, dtype=np.float32)
    R = n_nodes // n_cores
    assert R * n_cores == n_nodes
    G = (R + 127) // 128
    n_win = (n_nodes + win - 1) // win

    deg = np.zeros(n_nodes, dtype=np.float32)
    np.add.at(deg, dst, ew)
    with np.errstate(divide="ignore"):
        dis = np.where(deg > 0, 1.0 / np.sqrt(deg, dtype=np.float32), 0.0).astype(np.float32)
    norm_all = (dis[src] * ew * dis[dst]).astype(np.float32)

    n_keys = n_win * G
    counts = np.zeros((n_cores, n_keys), dtype=np.int64)
    per_core = []
    for k in range(n_cores):
        m = (dst >= k * R) & (dst < (k + 1) * R)
        s_k = src[m]
        d_k = dst[m] - k * R
        n_k = norm_all[m]
        w_k = s_k // win
        t_k = d_k // 128
        key = w_k * G + t_k
        np.add.at(counts[k], key, 1)
        per_core.append((s_k, d_k, n_k, key))

    # uniform chunks per (w, t): max over cores, at least 1
    nch = np.maximum(1, (counts.max(axis=0) + 127) // 128)  # [n_keys]
    chunk_base = np.zeros(n_keys + 1, dtype=np.int64)
    np.cumsum(nch, out=chunk_base[1:])
    total_chunks = int(chunk_base[-1])
    S = total_chunks * 128

    # calls: per window, runs of <= CCH chunks
    calls = []            # (w, chunk0, n_chunks)
    for w in range(n_win):
        c0 = int(chunk_base[w * G])
        c1 = int(chunk_base[(w + 1) * G])
        c = c0
        while c < c1:
            n = min(CCH, c1 - c)
            calls.append((w, c, n))
            c += n

    # per-core slot arrays
    idx16 = np.zeros((n_cores, 128, S // 16), dtype=np.int16)
    normp = np.zeros((n_cores, 128, S // 128), dtype=np.float32)
    dstbp = np.zeros((n_cores, 128, S // 128), dtype=ml_dtypes.bfloat16)
    slot_base = chunk_base * 128

    for k in range(n_cores):
        s_k, d_k, n_k, key = per_core[k]
        order = np.argsort(key, kind="stable")
        key_s = key[order]
        # within-group rank
        if key_s.size:
            starts = np.r_[0, np.flatnonzero(np.diff(key_s)) + 1]
            group_sizes = np.diff(np.r_[starts, key_s.size])
            rank = np.arange(key_s.size) - np.repeat(starts, group_sizes)
        else:
            rank = np.zeros(0, dtype=np.int64)
        slot = slot_base[key_s] + rank
        sv = np.zeros(S, dtype=np.int64)           # pad: idx 0 (valid row)
        nv = np.zeros(S, dtype=np.float32)         # pad: norm 0
        dv = np.zeros(S, dtype=np.int64)           # pad: dst_local 0
        sv[slot] = (s_k[order] - (s_k[order] // win) * win)
        nv[slot] = n_k[order]
        dv[slot] = d_k[order] - (d_k[order] // 128) * 128
        i = np.arange(S)
        a16 = np.zeros((16, S // 16), dtype=np.int16)
        a16[i % 16, i // 16] = sv.astype(np.int16)
        idx16[k] = np.tile(a16, (8, 1))
        normp[k][i % 128, i // 128] = nv
        dstbp[k][i % 128, i // 128] = dv.astype(ml_dtypes.bfloat16)

    return dict(R=R, G=G, S=S, n_win=n_win, nch=nch, chunk_base=chunk_base,
                calls=calls, idx16=idx16, norm=normp, dstb=dstbp)


# -------------------- device kernel --------------------

def _build_bass(plan_ii, plan_uiu, n_ii, n_uiu, n_users, d, n_layers, n_cores):
    from concourse import bacc, bass, mybir, tile

    nc = bacc.Bacc("TRN2", target_bir_lowering=False, debug=False,
                   num_devices=n_cores, num_swdge_queues=1)
    f32 = mybir.dt.float32
    bf16 = mybir.dt.bfloat16
    i16 = mybir.dt.int16

    emb_ii = nc.dram_tensor("emb_ii", [n_ii, d], f32, kind="ExternalInput")
    emb_uiu_user = nc.dram_tensor("emb_uiu_user", [n_users, d], f32, kind="ExternalInput")

    g_in = {}
    for gname, plan in (("ii", plan_ii), ("uiu", plan_uiu)):
        g_in[gname] = dict(
            idx16=nc.dram_tensor(f"idx16_{gname}", [128, plan["S"] // 16], i16, kind="ExternalInput"),
            norm=nc.dram_tensor(f"norm_{gname}", [128, plan["S"] // 128], f32, kind="ExternalInput"),
            dstb=nc.dram_tensor(f"dstb_{gname}", [128, plan["S"] // 128], bf16, kind="ExternalInput"),
        )

    R_uiu, G_uiu = plan_uiu["R"], plan_uiu["G"]
    R_ii, G_ii = plan_ii["R"], plan_ii["G"]

    h_out = nc.dram_tensor("h_out", [R_uiu, d], f32, kind="ExternalOutput")

    # internal DRAM
    tabA_ii = nc.dram_tensor("tabA_ii", [n_ii, d], f32)
    tabB_ii = nc.dram_tensor("tabB_ii", [n_ii, d], f32)
    x0_uiu = nc.dram_tensor("x0_uiu", [n_uiu, d], f32)
    tabB_uiu = nc.dram_tensor("tabB_uiu", [n_uiu, d], f32)
    hslice = nc.dram_tensor("hslice", [G_uiu * 128, d], f32)

    rg = [list(range(n_cores))]
    alpha = 1.0 / (n_layers + 1)

    with tile.TileContext(nc) as tc:
        with (
            tc.tile_pool(name="calls", bufs=2) as cpool,
            tc.tile_pool(name="big", bufs=1) as bigp,
            tc.tile_pool(name="psum", bufs=4, space="PSUM") as ppool,
        ):
            acc = bigp.tile([128, G_uiu, d], f32, tag="acc")
            out_acc = bigp.tile([128, G_uiu, d], f32, tag="out_acc")
            iota_b = bigp.tile([128, 128], bf16, tag="iota")
            nc.gpsimd.iota(iota_b[:], pattern=[[1, 128]], channel_multiplier=0,
                           allow_small_or_imprecise_dtypes=True)
            iota_bc = iota_b[:].rearrange("p (o f) -> p o f", o=1)

            pid = nc.sync.partition_id()

            def load_x0(dst_tile, table_ap, R, G):
                """dst_tile[:, 0:G, :] = rows [pid*R, pid*R+R) of table."""
                Gf = R // 128
                rows0 = Gf * 128
                rt = R - rows0
                base = pid * R
                nc.vector.memset(dst_tile[:, 0:G, :], 0.0)
                nc.sync.dma_start(
                    out=dst_tile[:, 0:Gf, :],
                    in_=table_ap[bass.ds(base, rows0), :].rearrange("(g p) d -> p g d", p=128))
                if rt:
                    nc.sync.dma_start(
                        out=dst_tile[0:rt, Gf, :],
                        in_=table_ap[bass.ds(base + rows0, rt), :].rearrange("(g p) d -> p g d", g=1))

            def run_layer(plan, gname, table_ap, n_nodes, G):
                """acc = one propagation of `table` along the graph."""
                nch = plan["nch"]
                chunk_base = plan["chunk_base"]
                calls = plan["calls"]
                n_win = plan["n_win"]
                nc.vector.memset(acc[:, 0:G, :], 0.0)

                call_idx = [-1]     # current call index
                call_tiles = [None]  # (V, S, chunk0, n)

                def ensure_call(c):
                    """Make sure chunk c's call is issued; return (V, S, j)."""
                    while call_idx[0] < 0 or c >= call_tiles[0][2] + call_tiles[0][3]:
                        call_idx[0] += 1
                        w, c0, n_chunks = calls[call_idx[0]]
                        n = n_chunks * 128
                        c16 = n // 16
                        pos = c0 * 128
                        gt = cpool.tile([128, CCH, d], f32, tag="g")
                        vt = cpool.tile([128, CCH, d], bf16, tag="v")
                        st = cpool.tile([128, CCH, 128], bf16, tag="s")
                        ist = cpool.tile([128, CCH * 8], i16, tag="is")
                        nt = cpool.tile([128, CCH], f32, tag="n")
                        dt = cpool.tile([128, CCH], bf16, tag="dt")
                        nc.sync.dma_start(out=ist[:, 0:c16],
                                          in_=g_in[gname]["idx16"][:, pos // 16: pos // 16 + c16])
                        nc.sync.dma_start(out=nt[:, 0:n_chunks],
                                          in_=g_in[gname]["norm"][:, c0: c0 + n_chunks])
                        nc.sync.dma_start(out=dt[:, 0:n_chunks],
                                          in_=g_in[gname]["dstb"][:, c0: c0 + n_chunks])
                        win_rows = min(WIN, n_nodes - w * WIN)
                        nc.gpsimd.dma_gather(
                            gt[:, 0:n_chunks, :],
                            table_ap[w * WIN: w * WIN + win_rows, :],
                            ist[:, 0:c16],
                            n, n, d,
                            single_packet=False,
                            queue_num=0,
                        )
                        # V = gathered * norm (cast to bf16)
                        nc.vector.tensor_tensor(
                            out=vt[:, 0:n_chunks, :],
                            in0=gt[:, 0:n_chunks, :],
                            in1=nt[:, 0:n_chunks].to_broadcast([128, n_chunks, d]),
                            op=mybir.AluOpType.mult,
                        )
                        # S[p, j, f] = (dst_local[p, j] == f)
                        nc.vector.tensor_tensor(
                            out=st[:, 0:n_chunks, :],
                            in0=dt[:, 0:n_chunks].to_broadcast([128, n_chunks, 128]),
                            in1=iota_bc.to_broadcast([128, n_chunks, 128]),
                            op=mybir.AluOpType.is_equal,
                        )
                        call_tiles[0] = (vt, st, c0, n_chunks)
                    vt, st, c0, n_chunks = call_tiles[0]
                    return vt, st, c - c0

                for w in range(n_win):
                    t0 = 0
                    while t0 < G:
                        gs = min(GTILES, G - t0)
                        pg = ppool.tile([128, GTILES, d], f32, tag="pg", space="PSUM")
                        # one accumulation group per PSUM bank: start zero-marks
                        # the whole 2KB zero region, each element's first write
                        # overwrites, later writes accumulate
                        n_mm = sum(int(nch[w * G + t0 + ti]) for ti in range(gs))
                        mm = 0
                        for ti in range(gs):
                            t = t0 + ti
                            key = w * G + t
                            c_first = int(chunk_base[key])
                            k_n = int(nch[key])
                            for j in range(k_n):
                                vt, st, jloc = ensure_call(c_first + j)
                                nc.tensor.matmul(
                                    out=pg[:, ti, :],
                                    lhsT=st[:, jloc, :],
                                    rhs=vt[:, jloc, :],
                                    start=(mm == 0),
                                    stop=(mm == n_mm - 1),
                                )
                                mm += 1
                        nc.vector.tensor_tensor(
                            out=acc[:, t0:t0 + gs, :],
                            in0=acc[:, t0:t0 + gs, :],
                            in1=pg[:, 0:gs, :],
                            op=mybir.AluOpType.add,
                        )
                        t0 += gs

            def store_slice(src_tile, R, G):
                """hslice[0:G*128] = src_tile rows (row r = g*128 + p)."""
                nc.sync.dma_start(
                    out=hslice[0:G * 128, :].rearrange("(g p) d -> p g d", p=128),
                    in_=src_tile[:, 0:G, :])

            # ---------------- graph ii ----------------
            load_x0(out_acc, emb_ii, R_ii, G_ii)
            tabs_ii = [emb_ii, tabA_ii, tabB_ii]
            for l in range(n_layers):
                run_layer(plan_ii, "ii", tabs_ii[l], n_ii, G_ii)
                nc.vector.tensor_tensor(out=out_acc[:, 0:G_ii, :], in0=out_acc[:, 0:G_ii, :],
                                        in1=acc[:, 0:G_ii, :], op=mybir.AluOpType.add)
                if l + 1 < n_layers:
                    store_slice(acc, R_ii, G_ii)
                    nc.gpsimd.collective_compute(
                        "AllGather", mybir.AluOpType.bypass, replica_groups=rg,
                        ins=[hslice[0:R_ii, :].opt()],
                        outs=[tabs_ii[l + 1][0:n_ii, :].opt()])

            # h_ii = alpha * out_acc -> AllGather into item block of x0_uiu
            nc.vector.tensor_scalar(out=out_acc[:, 0:G_ii, :], in0=out_acc[:, 0:G_ii, :],
                                    scalar1=alpha, scalar2=None, op0=mybir.AluOpType.mult)
            store_slice(out_acc, R_ii, G_ii)
            nc.gpsimd.collective_compute(
                "AllGather", mybir.AluOpType.bypass, replica_groups=rg,
                ins=[hslice[0:R_ii, :].opt()],
                outs=[x0_uiu[n_users:n_users + n_ii, :].opt()])

            # user half of the spliced table
            nc.sync.dma_start(out=x0_uiu[0:n_users, :], in_=emb_uiu_user[:, :])

            # ---------------- graph uiu ----------------
            load_x0(out_acc, x0_uiu, R_uiu, G_uiu)
            tabs_uiu = [x0_uiu, tabB_uiu, x0_uiu]
            for l in range(n_layers):
                run_layer(plan_uiu, "uiu", tabs_uiu[l], n_uiu, G_uiu)
                nc.vector.tensor_tensor(out=out_acc[:, 0:G_uiu, :], in0=out_acc[:, 0:G_uiu, :],
                                        in1=acc[:, 0:G_uiu, :], op=mybir.AluOpType.add)
                if l + 1 < n_layers:
                    store_slice(acc, R_uiu, G_uiu)
                    nc.gpsimd.collective_compute(
                        "AllGather", mybir.AluOpType.bypass, replica_groups=rg,
                        ins=[hslice[0:R_uiu, :].opt()],
                        outs=[tabs_uiu[l + 1][0:n_uiu, :].opt()])

            # h_uiu slice = alpha * out_acc -> h_out
            nc.vector.tensor_scalar(out=out_acc[:, 0:G_uiu, :], in0=out_acc[:, 0:G_uiu, :],
                                    scalar1=alpha, scalar2=None, op0=mybir.AluOpType.mult)
            Gf = R_uiu // 128
            rows0 = Gf * 128
            rt = R_uiu - rows0
            nc.sync.dma_start(
                out=h_out[0:rows0, :].rearrange("(g p) d -> p g d", p=128),
                in_=out_acc[:, 0:Gf, :])
            if rt:
                nc.sync.dma_start(
                    out=h_out[rows0:R_uiu, :].rearrange("(g p) d -> p g d", g=1),
                    in_=out_acc[0:rt, Gf, :])

    nc.compile()
    return nc


# -------------------- entry point --------------------

_CACHE = {}


def kernel(emb_ii, emb_uiu, edge_attr_ii, edge_attr_uiu,
           edge_index_ii, edge_index_uiu, item_idx):
    global _LAST_RESULT
    from concourse.bass_utils import run_bass_kernel_spmd

    emb_ii = np.asarray(emb_ii, dtype=np.float32)
    emb_uiu = np.asarray(emb_uiu, dtype=np.float32)
    item_idx = np.asarray(item_idx)
    assert np.array_equal(item_idx, np.arange(N_II, dtype=item_idx.dtype) + N_USERS), \
        "kernel assumes contiguous item block"

    key = "plan"
    if key not in _CACHE:
        plan_ii = _plan_graph(np.asarray(edge_index_ii[0]), np.asarray(edge_index_ii[1]),
                              np.asarray(edge_attr_ii), N_II)
        plan_uiu = _plan_graph(np.asarray(edge_index_uiu[0]), np.asarray(edge_index_uiu[1]),
                               np.asarray(edge_attr_uiu), N_UIU)
        nc = _build_bass(plan_ii, plan_uiu, N_II, N_UIU, N_USERS, D, L, N_CORES)
        _CACHE[key] = (plan_ii, plan_uiu, nc)
    plan_ii, plan_uiu, nc = _CACHE[key]

    in_maps = []
    for k in range(N_CORES):
        in_maps.append({
            "emb_ii": emb_ii,
            "emb_uiu_user": emb_uiu[:N_USERS],
            "idx16_ii": plan_ii["idx16"][k],
            "norm_ii": plan_ii["norm"][k],
            "dstb_ii": plan_ii["dstb"][k],
            "idx16_uiu": plan_uiu["idx16"][k],
            "norm_uiu": plan_uiu["norm"][k],
            "dstb_uiu": plan_uiu["dstb"][k],
        })

    res = run_bass_kernel_spmd(nc, in_maps, core_ids=list(range(N_CORES)))
    _LAST_RESULT = res
    out = np.concatenate([res.results[k]["h_out"] for k in range(N_CORES)], axis=0)
    return out.astype(np.float32)


# revision 12
# speedup vs baseline: 2.1550x; 2.1550x over previous
"""Trainium2 Bass kernel for BigraphLightModel (two-stage LightGCN).

Strategy (8 NeuronCores, SPMD), v2 — gather + TensorE segment-sum:
- Nodes of each graph are partitioned contiguously across the 8 cores
  (core k owns rows [k*N/8, (k+1)*N/8)); edges live on the core that owns
  their dst node. Each layer the full embedding table is rebuilt on every
  core by an AllGather (HBM->HBM), and each core dma_gathers the source
  rows of its edges from its local copy.
- The scatter-add is NOT done with dma_scatter_add (SWDGE descriptor
  generation on the Q7 cores was the 12ms bottleneck of v1). Instead,
  edges are laid out sorted by (src-window, dst-tile) where dst-tile is a
  128-row block of the core's node slice. For each 128-slot chunk the
  kernel builds a one-hot selection matrix S[slot, dst_local] on the
  Vector engine (is_equal against an iota row) and computes
  psum[dst, :] += S.T @ (norm * gathered) on the Tensor engine. PSUM
  tiles accumulate over a (window, tile) group and are added into an
  SBUF accumulator holding the core's [R, D] output slice.
- Slot layout is identical on all 8 cores (a (window,tile) group gets
  max-over-cores chunks, padded with norm=0 slots), so a single SPMD
  program works for every core.
- The per-layer output slice goes acc -> HBM -> AllGather -> next table.
  The item-block splice emb_uiu[item_idx] = h_ii is the AllGather of
  h_ii written into rows [N_USERS:N_USERS+N_II) of the uiu table.
"""
import os
import sys

for _p in ("/opt/trn_rl_repo",):
    if _p not in sys.path and os.path.isdir(_p):
        sys.path.insert(0, _p)

import numpy as np
import ml_dtypes

# -------------------- problem constants --------------------
N_II = 100000
N_UIU = 200000
D = 64
L = 3                   # layers per graph
N_CORES = 8
N_USERS = 100000
WIN_MAX = 32768         # int16 gather window limit (rows)
CCH = 32                # chunks per gather call (32*128 = 4096 slots)
GTILES = 8              # dst tiles per PSUM group (8*64 fp32 = one bank)

_LAST_RESULT = None     # test harness reads this for profiling info


# -------------------- host-side graph planning --------------------

def _plan_graph(src, dst, ew, n_nodes, n_cores=N_CORES):
    """Slot layout for one graph: slots sorted by (window, dst-tile),
    each (window, tile) padded to a uniform chunk count across cores.
    Windows are evenly sized (not WIN_MAX) so the per-(window,tile) edge
    count sits well below the 128-slot chunk boundary."""
    src = np.asarray(src).astype(np.int64)
    dst = np.asarray(dst).astype(np.int64)
    ew = np.asarray(ew, dtype=np.float32)
    R = n_nodes // n_cores
    assert R * n_cores == n_nodes
    G = (R + 127) // 128
    n_win = (n_nodes + WIN_MAX - 1) // WIN_MAX
    win = (n_nodes + n_win - 1) // n_win
    assert win <= WIN_MAX

    deg = np.zeros(n_nodes, dtype=np.float32)
    np.add.at(deg, dst, ew)
    with np.errstate(divide="ignore"):
        dis = np.where(deg > 0, 1.0 / np.sqrt(deg, dtype=np.float32), 0.0).astype(np.float32)
    norm_all = (dis[src] * ew * dis[dst]).astype(np.float32)

    n_keys = n_win * G
    counts = np.zeros((n_cores, n_keys), dtype=np.int64)
    per_core = []
    for k in range(n_cores):
        m = (dst >= k * R) & (dst < (k + 1) * R)
        s_k = src[m]
        d_k = dst[m] - k * R
        n_k = norm_all[m]
        w_k = s_k // win
        t_k = d_k // 128
        key = w_k * G + t_k
        np.add.at(counts[k], key, 1)
        per_core.append((s_k, d_k, n_k, key))

    # uniform chunks per (w, t): max over cores, at least 1
    nch = np.maximum(1, (counts.max(axis=0) + 127) // 128)  # [n_keys]
    chunk_base = np.zeros(n_keys + 1, dtype=np.int64)
    np.cumsum(nch, out=chunk_base[1:])
    total_chunks = int(chunk_base[-1])
    S = total_chunks * 128

    # calls: per window, runs of <= CCH chunks
    calls = []            # (w, chunk0, n_chunks)
    for w in range(n_win):
        c0 = int(chunk_base[w * G])
        c1 = int(chunk_base[(w + 1) * G])
        c = c0
        while c < c1:
            n = min(CCH, c1 - c)
            calls.append((w, c, n))
            c += n

    # per-core slot arrays
    idx16 = np.zeros((n_cores, 128, S // 16), dtype=np.int16)
    normp = np.zeros((n_cores, 128, S // 128), dtype=np.float32)
    dstbp = np.zeros((n_cores, 128, S // 128), dtype=np.float32)
    slot_base = chunk_base * 128

    for k in range(n_cores):
        s_k, d_k, n_k, key = per_core[k]
        order = np.argsort(key, kind="stable")
        key_s = key[order]
        # within-group rank
        if key_s.size:
            starts = np.r_[0, np.flatnonzero(np.diff(key_s)) + 1]
            group_sizes = np.diff(np.r_[starts, key_s.size])
            rank = np.arange(key_s.size) - np.repeat(starts, group_sizes)
        else:
            rank = np.zeros(0, dtype=np.int64)
        slot = slot_base[key_s] + rank
        sv = np.zeros(S, dtype=np.int64)           # pad: idx 0 (valid row)
        nv = np.zeros(S, dtype=np.float32)         # pad: norm 0
        dv = np.zeros(S, dtype=np.int64)           # pad: dst_local 0
        sv[slot] = (s_k[order] - (s_k[order] // win) * win)
        nv[slot] = n_k[order]
        dv[slot] = d_k[order] - (d_k[order] // 128) * 128
        i = np.arange(S)
        a16 = np.zeros((16, S // 16), dtype=np.int16)
        a16[i % 16, i // 16] = sv.astype(np.int16)
        idx16[k] = np.tile(a16, (8, 1))
        normp[k][i % 128, i // 128] = nv
        dstbp[k][i % 128, i // 128] = dv.astype(np.float32)

    return dict(R=R, G=G, S=S, n_win=n_win, win=win, nch=nch, chunk_base=chunk_base,
                calls=calls, idx16=idx16, norm=normp, dstb=dstbp)


# -------------------- device kernel --------------------

def _build_bass(plan_ii, plan_uiu, n_ii, n_uiu, n_users, d, n_layers, n_cores):
    from concourse import bacc, bass, mybir, tile

    nc = bacc.Bacc("TRN2", target_bir_lowering=False, debug=False,
                   num_devices=n_cores, num_swdge_queues=1)
    f32 = mybir.dt.float32
    i16 = mybir.dt.int16

    emb_ii = nc.dram_tensor("emb_ii", [n_ii, d], f32, kind="ExternalInput")
    emb_uiu_user = nc.dram_tensor("emb_uiu_user", [n_users, d], f32, kind="ExternalInput")

    g_in = {}
    for gname, plan in (("ii", plan_ii), ("uiu", plan_uiu)):
        g_in[gname] = dict(
            idx16=nc.dram_tensor(f"idx16_{gname}", [128, plan["S"] // 16], i16, kind="ExternalInput"),
            norm=nc.dram_tensor(f"norm_{gname}", [128, plan["S"] // 128], f32, kind="ExternalInput"),
            dstb=nc.dram_tensor(f"dstb_{gname}", [128, plan["S"] // 128], f32, kind="ExternalInput"),
        )

    R_uiu, G_uiu = plan_uiu["R"], plan_uiu["G"]
    R_ii, G_ii = plan_ii["R"], plan_ii["G"]

    h_out = nc.dram_tensor("h_out", [R_uiu, d], f32, kind="ExternalOutput")

    # internal DRAM
    tabA_ii = nc.dram_tensor("tabA_ii", [n_ii, d], f32)
    tabB_ii = nc.dram_tensor("tabB_ii", [n_ii, d], f32)
    x0_uiu = nc.dram_tensor("x0_uiu", [n_uiu, d], f32)
    tabB_uiu = nc.dram_tensor("tabB_uiu", [n_uiu, d], f32)
    hslice = nc.dram_tensor("hslice", [G_uiu * 128, d], f32)

    rg = [list(range(n_cores))]
    alpha = 1.0 / (n_layers + 1)

    with tile.TileContext(nc) as tc:
        with (
            tc.tile_pool(name="calls", bufs=2) as cpool,
            tc.tile_pool(name="big", bufs=1) as bigp,
            tc.tile_pool(name="psum", bufs=4, space="PSUM") as ppool,
        ):
            acc = bigp.tile([128, G_uiu, d], f32, tag="acc")
            out_acc = bigp.tile([128, G_uiu, d], f32, tag="out_acc")
            # iota replicated along the chunk dim so the S-build compare reads
            # a plain contiguous operand (keeps the DVE in its fast mode)
            iota_f = bigp.tile([128, CCH, 128], f32, tag="iota")
            nc.gpsimd.iota(iota_f[:], pattern=[[0, CCH], [1, 128]], channel_multiplier=0,
                           allow_small_or_imprecise_dtypes=True)

            pid = nc.sync.partition_id()

            def load_x0(dst_tile, table_ap, R, G):
                """dst_tile[:, 0:G, :] = rows [pid*R, pid*R+R) of table."""
                Gf = R // 128
                rows0 = Gf * 128
                rt = R - rows0
                base = pid * R
                nc.vector.memset(dst_tile[:, 0:G, :], 0.0)
                nc.sync.dma_start(
                    out=dst_tile[:, 0:Gf, :],
                    in_=table_ap[bass.ds(base, rows0), :].rearrange("(g p) d -> p g d", p=128))
                if rt:
                    nc.sync.dma_start(
                        out=dst_tile[0:rt, Gf, :],
                        in_=table_ap[bass.ds(base + rows0, rt), :].rearrange("(g p) d -> p g d", g=1))

            def run_layer(plan, gname, table_ap, n_nodes, G):
                """acc = one propagation of `table` along the graph."""
                nch = plan["nch"]
                chunk_base = plan["chunk_base"]
                calls = plan["calls"]
                n_win = plan["n_win"]
                nc.vector.memset(acc[:, 0:G, :], 0.0)

                call_idx = [-1]     # current call index
                call_tiles = [None]  # (V, S, chunk0, n)

                win = plan["win"]

                def ensure_call(c):
                    """Make sure chunk c's call is issued; return (V, S, j)."""
                    while call_idx[0] < 0 or c >= call_tiles[0][2] + call_tiles[0][3]:
                        call_idx[0] += 1
                        w, c0, n_chunks = calls[call_idx[0]]
                        n = n_chunks * 128
                        c16 = n // 16
                        pos = c0 * 128
                        gt = cpool.tile([128, CCH, d], f32, tag="g")
                        st = cpool.tile([128, CCH, 128], f32, tag="s")
                        ist = cpool.tile([128, CCH * 8], i16, tag="is")
                        nt = cpool.tile([128, CCH], f32, tag="n")
                        dt = cpool.tile([128, CCH], f32, tag="dt")
                        nc.sync.dma_start(out=ist[:, 0:c16],
                                          in_=g_in[gname]["idx16"][:, pos // 16: pos // 16 + c16])
                        nc.sync.dma_start(out=nt[:, 0:n_chunks],
                                          in_=g_in[gname]["norm"][:, c0: c0 + n_chunks])
                        nc.sync.dma_start(out=dt[:, 0:n_chunks],
                                          in_=g_in[gname]["dstb"][:, c0: c0 + n_chunks])
                        win_rows = min(win, n_nodes - w * win)
                        nc.gpsimd.dma_gather(
                            gt[:, 0:n_chunks, :],
                            table_ap[w * win: w * win + win_rows, :],
                            ist[:, 0:c16],
                            n, n, d,
                            single_packet=False,
                            queue_num=0,
                        )
                        # V = gathered * norm (in place, same op shape v1 ran at 4x)
                        nc.vector.tensor_tensor(
                            out=gt[:, 0:n_chunks, :],
                            in0=gt[:, 0:n_chunks, :],
                            in1=nt[:, 0:n_chunks].to_broadcast([128, n_chunks, d]),
                            op=mybir.AluOpType.mult,
                        )
                        # S[p, j, f] = (dst_local[p, j] == f)
                        nc.vector.tensor_tensor(
                            out=st[:, 0:n_chunks, :],
                            in0=dt[:, 0:n_chunks].to_broadcast([128, n_chunks, 128]),
                            in1=iota_f[:, 0:n_chunks, :],
                            op=mybir.AluOpType.is_equal,
                        )
                        call_tiles[0] = (gt, st, c0, n_chunks)
                    vt, st, c0, n_chunks = call_tiles[0]
                    return vt, st, c - c0

                for w in range(n_win):
                    t0 = 0
                    while t0 < G:
                        gs = min(GTILES, G - t0)
                        pg = ppool.tile([128, GTILES, d], f32, tag="pg", space="PSUM")
                        # one accumulation group per PSUM bank: start zero-marks
                        # the whole 2KB zero region, each element's first write
                        # overwrites, later writes accumulate
                        n_mm = sum(int(nch[w * G + t0 + ti]) for ti in range(gs))
                        mm = 0
                        for ti in range(gs):
                            t = t0 + ti
                            key = w * G + t
                            c_first = int(chunk_base[key])
                            k_n = int(nch[key])
                            for j in range(k_n):
                                vt, st, jloc = ensure_call(c_first + j)
                                nc.tensor.matmul(
                                    out=pg[:, ti, :],
                                    lhsT=st[:, jloc, :],
                                    rhs=vt[:, jloc, :],
                                    start=(mm == 0),
                                    stop=(mm == n_mm - 1),
                                )
                                mm += 1
                        nc.vector.tensor_tensor(
                            out=acc[:, t0:t0 + gs, :],
                            in0=acc[:, t0:t0 + gs, :],
                            in1=pg[:, 0:gs, :],
                            op=mybir.AluOpType.add,
                        )
                        t0 += gs

            def store_slice(src_tile, R, G):
                """hslice[0:G*128] = src_tile rows (row r = g*128 + p)."""
                nc.sync.dma_start(
                    out=hslice[0:G * 128, :].rearrange("(g p) d -> p g d", p=128),
                    in_=src_tile[:, 0:G, :])

            # ---------------- graph ii ----------------
            load_x0(out_acc, emb_ii, R_ii, G_ii)
            tabs_ii = [emb_ii, tabA_ii, tabB_ii]
            for l in range(n_layers):
                run_layer(plan_ii, "ii", tabs_ii[l], n_ii, G_ii)
                nc.vector.tensor_tensor(out=out_acc[:, 0:G_ii, :], in0=out_acc[:, 0:G_ii, :],
                                        in1=acc[:, 0:G_ii, :], op=mybir.AluOpType.add)
                if l + 1 < n_layers:
                    store_slice(acc, R_ii, G_ii)
                    nc.gpsimd.collective_compute(
                        "AllGather", mybir.AluOpType.bypass, replica_groups=rg,
                        ins=[hslice[0:R_ii, :].opt()],
                        outs=[tabs_ii[l + 1][0:n_ii, :].opt()])

            # h_ii = alpha * out_acc -> AllGather into item block of x0_uiu
            nc.vector.tensor_scalar(out=out_acc[:, 0:G_ii, :], in0=out_acc[:, 0:G_ii, :],
                                    scalar1=alpha, scalar2=None, op0=mybir.AluOpType.mult)
            store_slice(out_acc, R_ii, G_ii)
            nc.gpsimd.collective_compute(
                "AllGather", mybir.AluOpType.bypass, replica_groups=rg,
                ins=[hslice[0:R_ii, :].opt()],
                outs=[x0_uiu[n_users:n_users + n_ii, :].opt()])

            # user half of the spliced table
            nc.sync.dma_start(out=x0_uiu[0:n_users, :], in_=emb_uiu_user[:, :])

            # ---------------- graph uiu ----------------
            load_x0(out_acc, x0_uiu, R_uiu, G_uiu)
            tabs_uiu = [x0_uiu, tabB_uiu, x0_uiu]
            for l in range(n_layers):
                run_layer(plan_uiu, "uiu", tabs_uiu[l], n_uiu, G_uiu)
                nc.vector.tensor_tensor(out=out_acc[:, 0:G_uiu, :], in0=out_acc[:, 0:G_uiu, :],
                                        in1=acc[:, 0:G_uiu, :], op=mybir.AluOpType.add)
                if l + 1 < n_layers:
                    store_slice(acc, R_uiu, G_uiu)
                    nc.gpsimd.collective_compute(
                        "AllGather", mybir.AluOpType.bypass, replica_groups=rg,
                        ins=[hslice[0:R_uiu, :].opt()],
                        outs=[tabs_uiu[l + 1][0:n_uiu, :].opt()])

            # h_uiu slice = alpha * out_acc -> h_out
            nc.vector.tensor_scalar(out=out_acc[:, 0:G_uiu, :], in0=out_acc[:, 0:G_uiu, :],
                                    scalar1=alpha, scalar2=None, op0=mybir.AluOpType.mult)
            Gf = R_uiu // 128
            rows0 = Gf * 128
            rt = R_uiu - rows0
            nc.sync.dma_start(
                out=h_out[0:rows0, :].rearrange("(g p) d -> p g d", p=128),
                in_=out_acc[:, 0:Gf, :])
            if rt:
                nc.sync.dma_start(
                    out=h_out[rows0:R_uiu, :].rearrange("(g p) d -> p g d", g=1),
                    in_=out_acc[0:rt, Gf, :])

    nc.compile()
    return nc


# -------------------- entry point --------------------

_CACHE = {}


def kernel(emb_ii, emb_uiu, edge_attr_ii, edge_attr_uiu,
           edge_index_ii, edge_index_uiu, item_idx):
    global _LAST_RESULT
    from concourse.bass_utils import run_bass_kernel_spmd

    emb_ii = np.asarray(emb_ii, dtype=np.float32)
    emb_uiu = np.asarray(emb_uiu, dtype=np.float32)
    item_idx = np.asarray(item_idx)
    assert np.array_equal(item_idx, np.arange(N_II, dtype=item_idx.dtype) + N_USERS), \
        "kernel assumes contiguous item block"

    key = "plan"
    if key not in _CACHE:
        plan_ii = _plan_graph(np.asarray(edge_index_ii[0]), np.asarray(edge_index_ii[1]),
                              np.asarray(edge_attr_ii), N_II)
        plan_uiu = _plan_graph(np.asarray(edge_index_uiu[0]), np.asarray(edge_index_uiu[1]),
                               np.asarray(edge_attr_uiu), N_UIU)
        nc = _build_bass(plan_ii, plan_uiu, N_II, N_UIU, N_USERS, D, L, N_CORES)
        _CACHE[key] = (plan_ii, plan_uiu, nc)
    plan_ii, plan_uiu, nc = _CACHE[key]

    in_maps = []
    for k in range(N_CORES):
        in_maps.append({
            "emb_ii": emb_ii,
            "emb_uiu_user": emb_uiu[:N_USERS],
            "idx16_ii": plan_ii["idx16"][k],
            "norm_ii": plan_ii["norm"][k],
            "dstb_ii": plan_ii["dstb"][k],
            "idx16_uiu": plan_uiu["idx16"][k],
            "norm_uiu": plan_uiu["norm"][k],
            "dstb_uiu": plan_uiu["dstb"][k],
        })

    res = run_bass_kernel_spmd(nc, in_maps, core_ids=list(range(N_CORES)))
    _LAST_RESULT = res
    out = np.concatenate([res.results[k]["h_out"] for k in range(N_CORES)], axis=0)
    return out.astype(np.float32)
